# revision 11
# baseline (speedup 1.0000x reference)
"""Trainium2 Bass kernel for FFNWithScales (SwiGLU MLP with low-rank dequant scales).

Reference computation (all fp32):
    gate_eff = gate_snapped * (gate_scale_A @ gate_scale_B)       # [8192, 2048]
    up_eff   = up_snapped   * (up_scale_A   @ up_scale_B)         # [8192, 2048]
    down_eff = down_snapped * (down_scale_A @ down_scale_B)       # [2048, 8192]
    h   = silu(gate_eff @ x) * (up_eff @ x)                       # [8192, 512]
    out = down_eff @ h                                            # [2048, 512]

Sharding (8 cores, tensor-parallel on d_ff): core c owns d_ff rows
[c*1024, (c+1)*1024) of gate/up (and the matching columns of down).
Each core computes a full-[2048, 512] partial of the down projection;
partials are summed on the host (the all-reduce step).

Schedule notes (the PE is the pacing engine at ~2.4 GHz / 1 row/cycle bf16):
  - Weights stream as [128, 2, 512] fp32 pairs; a rank-32 scale matmul
    produces the matching scale tile in psum, DVE multiplies (fp32*fp32
    -> bf16), then eight [128,128]x[128,512] bf16 mains accumulate.
  - Scale matmuls for TWO pairs (4 chunks) are packed into ONE PE slot
    via 4-way tile_position quadrants (rows 0/32/64/96, K=32 each), so
    the PE pays the array-reconfig transition half as often.
  - The last two pairs of every pass issue fi-major so each psum
    accumulator finishes ~4 matmuls before the next, letting the
    serial epilogue (ACT silu / Pool muls / copies) chase the tail and
    release accumulators with no pass-boundary bubble. Up-pass muls run
    on Pool only: DVE must stay clear for the next pass's dequants.
  - Startup: job 0's weight DMA is split into two single-chunk DMAs on
    opposite rings with chunk-level dequant/mains so the PE starts
    ~1.5us earlier; factors split across rings; x streams on the
    gpsimd SWDGE ring at chunk granularity.
  - Output is stored bf16 (partials summed fp32 on host): halves store
    traffic and the kernel tail for ~1e-3 extra relative error.
"""

import numpy as np
import ml_dtypes

import concourse.bass as bass
from concourse import bacc
import concourse.mybir as mybir
from concourse.tile import TileContext
from concourse.bass_utils import run_bass_kernel_spmd

P = 128
D = 2048        # d_model
FF = 8192       # d_ff (global)
S = 512         # sequence
R = 32          # rank
NCORES = 8
F = FF // NCORES          # 1024 local d_ff rows
KD = D // P               # 16 d_model chunks
KF = F // P               # 8 local d_ff chunks
FG = 512                  # free-dim group (psum bank width)

f32 = mybir.dt.float32
bf16 = mybir.dt.bfloat16

_CACHE = {}


def _build():
    nc = bacc.Bacc()
    # Everything arrives bf16 in device layout (host staging); the dequant
    # multiply against the fp32 psum scale tile restores scale precision.
    x = nc.declare_dram_parameter("x", [D, S], bf16, isOutput=False)
    gT = nc.declare_dram_parameter("gT", [D, F], bf16, isOutput=False)
    uT = nc.declare_dram_parameter("uT", [D, F], bf16, isOutput=False)
    dT = nc.declare_dram_parameter("dT", [F, D], bf16, isOutput=False)
    # B4 [128, nk/4, 128]: strip i (partitions 32i..32i+31) holds B cols for
    # chunk 4*g+i (lhsT of the 4-packed scale matmul); AT4 [128, w]: A^T
    # replicated on all four strips.
    gB4 = nc.declare_dram_parameter("gB4", [4 * R, KD // 4, P], bf16, isOutput=False)
    uB4 = nc.declare_dram_parameter("uB4", [4 * R, KD // 4, P], bf16, isOutput=False)
    dB4 = nc.declare_dram_parameter("dB4", [4 * R, KF // 4, P], bf16, isOutput=False)
    gAT4 = nc.declare_dram_parameter("gAT4", [4 * R, F], bf16, isOutput=False)
    uAT4 = nc.declare_dram_parameter("uAT4", [4 * R, F], bf16, isOutput=False)
    dAT4 = nc.declare_dram_parameter("dAT4", [4 * R, D], bf16, isOutput=False)
    out = nc.declare_dram_parameter("out", [D, S], bf16, isOutput=True)

    with TileContext(nc) as tc:
        with (
            tc.tile_pool(name="const", bufs=1) as const,
            tc.tile_pool(name="wstream", bufs=12) as wpool,
            tc.tile_pool(name="hbuf", bufs=1) as hpool,
            tc.tile_pool(name="obuf", bufs=4) as opool,
            tc.tile_pool(name="psacc", bufs=1, space="PSUM") as psacc,
            tc.tile_pool(name="pssc", bufs=2, space="PSUM") as pssc,
        ):
            # Startup critical path: sc4(0) needs gB4 + the first 512-col
            # slice of gAT4 (AT factors load as per-pass slices so the first
            # scale matmul isn't gated on the full tensor); job 0's weight
            # chunks lead the scalar ring. x0/x1 ride the tails of the two
            # HWDGE rings; everything else takes the gpsimd SWDGE ring.
            rounded = {}

            def load_factor(nm, dram, eng):
                rt = const.tile(list(dram.shape), bf16, name=f"{nm}r", tag=f"{nm}r")
                eng.dma_start(rt, dram[:])
                rounded[nm] = rt

            def load_at_slice(nm, dram, sl, eng):
                rt = const.tile([4 * R, FG], bf16, name=f"{nm}{sl}", tag=f"{nm}{sl}")
                eng.dma_start(rt, dram[:, sl * FG:(sl + 1) * FG])
                rounded[nm, sl] = rt

            x_sb = [None] * KD

            def load_x_chunk(q, eng):
                xt = const.tile([P, S], bf16, name=f"x{q}", tag=f"x{q}")
                eng.dma_start(xt, x[q * P:(q + 1) * P, :])
                x_sb[q] = xt

            # sync: gate factors then odd weight pairs; scalar: x0 then job-0
            # weight chunks then even pairs; everything else on SWDGE.
            load_factor("gB", gB4, nc.sync)
            load_at_slice("gAT", gAT4, 0, nc.sync)
            load_x_chunk(0, nc.scalar)
            load_x_chunk(1, nc.gpsimd)
            load_x_chunk(2, nc.gpsimd)
            load_x_chunk(3, nc.gpsimd)

            load_at_slice("gAT", gAT4, 1, nc.gpsimd)
            load_factor("uB", uB4, nc.gpsimd)
            load_at_slice("uAT", uAT4, 0, nc.gpsimd)
            load_at_slice("uAT", uAT4, 1, nc.gpsimd)
            load_factor("dBs", dB4, nc.gpsimd)
            for mg in range(D // FG):
                load_at_slice("dAT", dAT4, mg, nc.gpsimd)

            def xs(kd):
                return x_sb[kd]

            # h = silu(gate) * up, [128, 8, 512] resident
            h_sb = hpool.tile([P, KF, S], bf16)

            silu = mybir.ActivationFunctionType.Silu

            def gate_finish(fi, fg, acc_fi):
                nc.scalar.activation(h_sb[:, fg * 4 + fi], acc_fi, silu)

            def up_finish(fi, fg, acc_fi):
                # DVE (Pool can't read PSUM); only 2 dequants precede these
                # in the DVE FIFO so they still chase the pass tail.
                nc.vector.tensor_mul(
                    out=h_sb[:, fg * 4 + fi], in0=h_sb[:, fg * 4 + fi],
                    in1=acc_fi)

            def down_finish(fi, mg, acc_fi):
                ot = opool.tile([P, S], bf16, name="ot", tag="ot")
                ceng = nc.scalar if fi % 2 == 0 else nc.vector
                if ceng is nc.scalar:
                    ceng.copy(ot, acc_fi)
                else:
                    ceng.tensor_copy(out=ot, in_=acc_fi)
                weng = nc.sync if fi % 2 == 0 else nc.scalar
                weng.dma_start(out[(mg * 4 + fi) * P:(mg * 4 + fi + 1) * P, :], ot)

            passes = []
            for is_up in (0, 1):
                for fg in range(F // FG):
                    passes.append(dict(
                        wdram=uT if is_up else gT,
                        Bn="uB" if is_up else "gB",
                        An="uAT" if is_up else "gAT",
                        nk=KD, fg=fg, rhs_fn=xs,
                        finish=(lambda fi, acc_fi, fg=fg, is_up=is_up:
                                up_finish(fi, fg, acc_fi) if is_up
                                else gate_finish(fi, fg, acc_fi)),
                    ))
            for mg in range(D // FG):
                passes.append(dict(
                    wdram=dT, Bn="dBs", An="dAT",
                    nk=KF, fg=mg, rhs_fn=lambda kf: h_sb[:, kf],
                    finish=lambda fi, acc_fi, mg=mg: down_finish(fi, mg, acc_fi),
                ))

            # Flat pair-job list. Every pass has an even number of pairs and
            # starts at an even flat index, so (E, E+1) groups for even E
            # never straddle a pass boundary.
            jobs = []
            for pi, ps in enumerate(passes):
                for kp in range(ps["nk"] // 2):
                    jobs.append((pi, kp))
            njobs = len(jobs)

            sc_tiles = {}

            def emit_sc4(E):
                """One 4-packed PE slot computing scale tiles for jobs E and
                E+1 (chunks 4g..4g+3 of pass pi)."""
                pi, kp = jobs[E]
                ps = passes[pi]
                fg = ps["fg"]
                g = kp // 2
                sca = pssc.tile([P, 2, FG], f32, name="sc", tag="sc")
                scb = pssc.tile([P, 2, FG], f32, name="sc", tag="sc")
                for i in range(4):
                    tgt = sca if i < 2 else scb
                    nc.tensor.matmul(
                        tgt[:, i % 2],
                        rounded[ps["Bn"]][i * R:(i + 1) * R, g],
                        rounded[ps["An"], fg][i * R:(i + 1) * R, :],
                        start=True, stop=True,
                        tile_position=(R * i, 0),
                    )
                sc_tiles[E] = sca
                sc_tiles[E + 1] = scb

            wt_tiles = {}
            wr_tiles = {}

            def emit_wt(J):
                """Weight DMA for pair J, issued several jobs ahead of the
                dequant so a slow transfer can't starve the PE. Job 0 splits
                into single-chunk DMAs on opposite rings so the first mains
                start as early as possible."""
                pi, kp = jobs[J]
                ps = passes[pi]
                fg = ps["fg"]
                if J == 0:
                    for j in range(2):
                        wt1 = wpool.tile([P, FG], bf16, name="wt0", tag=f"wt0{j}")
                        nc.scalar.dma_start(
                            wt1,
                            ps["wdram"][(2 * kp + j) * P:(2 * kp + j + 1) * P,
                                        fg * FG:(fg + 1) * FG])
                        wt_tiles[J, j] = wt1
                    return
                wt2 = wpool.tile([P, 2, FG], bf16, name="wt", tag="wt")
                weng = nc.sync if J % 2 == 1 else nc.scalar
                weng.dma_start(
                    wt2,
                    ps["wdram"][kp * 2 * P:(kp + 1) * 2 * P,
                                fg * FG:(fg + 1) * FG].rearrange(
                                    "(ko p) f -> p ko f", p=P))
                wt_tiles[J] = wt2

            def emit_dequant(J):
                if J == 0:
                    # chunk-granular so j=0 mains only wait on the first
                    # 256 KiB of weights
                    for j in range(2):
                        wr1 = wpool.tile([P, FG], bf16, name="wr0", tag=f"wr0{j}")
                        nc.vector.tensor_mul(
                            out=wr1, in0=wt_tiles.pop((J, j)),
                            in1=sc_tiles[J][:, j])
                        wr_tiles[J, j] = wr1
                    sc_tiles.pop(J)
                    return
                wr2 = wpool.tile([P, 2, FG], bf16, name="wr", tag="wr")
                nc.vector.tensor_mul(out=wr2, in0=wt_tiles.pop(J),
                                     in1=sc_tiles.pop(J))
                wr_tiles[J] = wr2

            def wr_slice(J, j, fi):
                if J == 0:
                    return wr_tiles[J, j][:, fi * P:(fi + 1) * P]
                return wr_tiles[J][:, j, fi * P:(fi + 1) * P]

            def free_wr(J):
                if J == 0:
                    wr_tiles.pop((J, 0))
                    wr_tiles.pop((J, 1))
                else:
                    wr_tiles.pop(J)

            DMA_AHEAD = 8
            for J in range(DMA_AHEAD):
                emit_wt(J)
            emit_sc4(0)
            emit_dequant(0)
            emit_dequant(1)

            cur_acc = {}
            for J, (pi, kp) in enumerate(jobs):
                ps = passes[pi]
                npairs = ps["nk"] // 2
                if kp == 0:
                    cur_acc[pi] = [
                        psacc.tile([P, S], f32, name=f"acc{i}", tag=f"acc{i}")
                        for i in range(4)]
                if pi == 0 and 2 * kp + 5 < KD:
                    # pull the rest of x in just-in-time on the SWDGE ring
                    # (chunk q is first consumed at pair kp=q//2)
                    load_x_chunk(2 * kp + 4, nc.gpsimd)
                    load_x_chunk(2 * kp + 5, nc.gpsimd)
                if J + DMA_AHEAD < njobs:
                    emit_wt(J + DMA_AHEAD)
                if J % 2 == 1 and J + 1 < njobs:
                    # sc4 for the next even group leads mains(J) in the PE
                    # FIFO: its dequants then overlap mains(J)/mains(J+1).
                    emit_sc4(J + 1)
                    emit_dequant(J + 1)
                    emit_dequant(J + 2)
                acc = cur_acc[pi]
                if kp == npairs - 2:
                    continue  # emitted fused with the last pair below
                if kp == npairs - 1:
                    # fi-major tail over the last two pairs: acc[fi] gets its
                    # final accumulation 4 matmuls after acc[fi-1], so the
                    # per-fi epilogue chases the tail.
                    for fi in range(4):
                        for Jt, kpt in ((J - 1, npairs - 2), (J, npairs - 1)):
                            for j in range(2):
                                nc.tensor.matmul(
                                    acc[fi],
                                    wr_slice(Jt, j, fi),
                                    ps["rhs_fn"](2 * kpt + j),
                                    start=False,
                                    stop=(kpt == npairs - 1 and j == 1),
                                )
                        ps["finish"](fi, acc[fi])
                    free_wr(J - 1)
                    free_wr(J)
                    cur_acc.pop(pi)
                    continue
                for j in range(2):
                    for fi in range(4):
                        nc.tensor.matmul(
                            acc[fi],
                            wr_slice(J, j, fi),
                            ps["rhs_fn"](2 * kp + j),
                            start=(kp == 0 and j == 0),
                            stop=False,
                        )
                free_wr(J)
    nc.finalize()
    return nc


def _prep_inputs(x, gate_snapped, gate_scale_A, gate_scale_B,
                 up_snapped, up_scale_A, up_scale_B,
                 down_snapped, down_scale_A, down_scale_B):
    asf = lambda a: np.ascontiguousarray(np.asarray(a, dtype=np.float32))
    bf = ml_dtypes.bfloat16
    x2 = np.ascontiguousarray(np.asarray(x, dtype=np.float32).reshape(D, S)
                              .astype(bf))
    gT_full = asf(gate_snapped).T      # [D, FF] view
    uT_full = asf(up_snapped).T
    dT_full = asf(down_snapped).T      # [FF, D] view

    def pack_B4(Bmat, nk):
        # [R, nk*128] -> [128, nk/4, 128]: strip i holds chunks 4*g+i
        b = np.asarray(Bmat, dtype=np.float32).reshape(R, nk // 4, 4, P)
        o = np.empty((4 * R, nk // 4, P), dtype=bf)
        for i in range(4):
            o[i * R:(i + 1) * R] = b[:, :, i, :].astype(bf)
        return o

    def pack_AT4(Amat):
        # A [w, R] -> A^T [R, w] replicated on all four strips -> [128, w]
        at = np.asarray(Amat, dtype=np.float32).T.astype(bf)
        return np.ascontiguousarray(np.concatenate([at] * 4, axis=0))

    gB_f = np.asarray(gate_scale_B, dtype=np.float32)
    uB_f = np.asarray(up_scale_B, dtype=np.float32)
    dB_f = np.asarray(down_scale_B, dtype=np.float32)
    gA_f = np.asarray(gate_scale_A, dtype=np.float32)
    uA_f = np.asarray(up_scale_A, dtype=np.float32)
    dAT4 = pack_AT4(down_scale_A)      # [128, D]

    in_maps = []
    for c in range(NCORES):
        lo, hi = c * F, (c + 1) * F
        in_maps.append({
            "x": x2,
            "gT": gT_full[:, lo:hi].astype(bf),
            "uT": uT_full[:, lo:hi].astype(bf),
            "dT": dT_full[lo:hi, :].astype(bf),
            "gB4": pack_B4(gB_f, KD),
            "uB4": pack_B4(uB_f, KD),
            "dB4": pack_B4(dB_f[:, lo:hi], KF),
            "gAT4": pack_AT4(gA_f[lo:hi]),
            "uAT4": pack_AT4(uA_f[lo:hi]),
            "dAT4": dAT4,
        })
    return in_maps


def run(trace=False, **inputs):
    if "nc" not in _CACHE:
        _CACHE["nc"] = _build()
    nc = _CACHE["nc"]
    in_maps = _prep_inputs(**inputs)
    try:
        res = run_bass_kernel_spmd(nc, in_maps, list(range(NCORES)), trace=trace)
    except Exception:
        # A transient device flake (NRT_EXEC_UNIT_UNRECOVERABLE) poisons the
        # PJRT client for the process; tearing the backend down and
        # reconnecting recovers it the same way a fresh process does.
        try:
            import jax.extend.backend
            jax.extend.backend.clear_backends()
        except Exception:
            pass
        res = run_bass_kernel_spmd(nc, in_maps, list(range(NCORES)), trace=trace)
    partial = np.zeros((D, S), dtype=np.float32)
    for c in range(NCORES):
        partial += np.asarray(res.results[c]["out"], dtype=np.float32)
    return partial.reshape(1, D, 1, S), res


def kernel(**inputs):
    out, _ = run(trace=False, **inputs)
    return out


if __name__ == "__main__":
    rng = np.random.default_rng(0)
    ins = {
        "x": rng.standard_normal((1, D, 1, S)).astype(np.float32),
        "gate_snapped": (rng.standard_normal((FF, D)) * 0.02).astype(np.float32),
        "gate_scale_A": (rng.standard_normal((FF, R)) * 0.1).astype(np.float32),
        "gate_scale_B": (rng.standard_normal((R, D)) * 0.1).astype(np.float32),
        "up_snapped": (rng.standard_normal((FF, D)) * 0.02).astype(np.float32),
        "up_scale_A": (rng.standard_normal((FF, R)) * 0.1).astype(np.float32),
        "up_scale_B": (rng.standard_normal((R, D)) * 0.1).astype(np.float32),
        "down_snapped": (rng.standard_normal((D, FF)) * 0.02).astype(np.float32),
        "down_scale_A": (rng.standard_normal((D, R)) * 0.1).astype(np.float32),
        "down_scale_B": (rng.standard_normal((R, FF)) * 0.1).astype(np.float32),
    }
    out = kernel(**ins)
    print("kernel ran, out shape", out.shape, "mean abs", np.abs(out).mean())


# revision 16
# speedup vs baseline: 1.0117x; 1.0117x over previous
"""Trainium2 Bass kernel for FFNWithScales (SwiGLU MLP with low-rank dequant scales).

Reference computation (all fp32):
    gate_eff = gate_snapped * (gate_scale_A @ gate_scale_B)       # [8192, 2048]
    up_eff   = up_snapped   * (up_scale_A   @ up_scale_B)         # [8192, 2048]
    down_eff = down_snapped * (down_scale_A @ down_scale_B)       # [2048, 8192]
    h   = silu(gate_eff @ x) * (up_eff @ x)                       # [8192, 512]
    out = down_eff @ h                                            # [2048, 512]

Sharding (8 cores, tensor-parallel on d_ff): core c owns d_ff rows
[c*1024, (c+1)*1024) of gate/up (and the matching columns of down).
Each core computes a full-[2048, 512] partial of the down projection;
partials are summed on the host (the all-reduce step).

Schedule notes (the PE is the pacing engine at ~2.4 GHz / 1 row/cycle bf16):
  - Weights stream as [128, 2, 512] fp32 pairs; a rank-32 scale matmul
    produces the matching scale tile in psum, DVE multiplies (fp32*fp32
    -> bf16), then eight [128,128]x[128,512] bf16 mains accumulate.
  - Scale matmuls for TWO pairs (4 chunks) are packed into ONE PE slot
    via 4-way tile_position quadrants (rows 0/32/64/96, K=32 each), so
    the PE pays the array-reconfig transition half as often.
  - The last two pairs of every pass issue fi-major so each psum
    accumulator finishes ~4 matmuls before the next, letting the
    serial epilogue (ACT silu / Pool muls / copies) chase the tail and
    release accumulators with no pass-boundary bubble. Up-pass muls run
    on Pool only: DVE must stay clear for the next pass's dequants.
  - Startup: job 0's weight DMA is split into two single-chunk DMAs on
    opposite rings with chunk-level dequant/mains so the PE starts
    ~1.5us earlier; factors split across rings; x streams on the
    gpsimd SWDGE ring at chunk granularity.
  - Output is stored bf16 (partials summed fp32 on host): halves store
    traffic and the kernel tail for ~1e-3 extra relative error.
"""

import numpy as np
import ml_dtypes

import concourse.bass as bass
from concourse import bacc
import concourse.mybir as mybir
from concourse.tile import TileContext
from concourse.bass_utils import run_bass_kernel_spmd

P = 128
D = 2048        # d_model
FF = 8192       # d_ff (global)
S = 512         # sequence
R = 32          # rank
NCORES = 8
F = FF // NCORES          # 1024 local d_ff rows
KD = D // P               # 16 d_model chunks
KF = F // P               # 8 local d_ff chunks
FG = 512                  # free-dim group (psum bank width)

f32 = mybir.dt.float32
bf16 = mybir.dt.bfloat16

_CACHE = {}


def _build():
    nc = bacc.Bacc()
    # Everything arrives bf16 in device layout (host staging); the dequant
    # multiply against the fp32 psum scale tile restores scale precision.
    x = nc.declare_dram_parameter("x", [D, S], bf16, isOutput=False)
    gT = nc.declare_dram_parameter("gT", [D, F], bf16, isOutput=False)
    uT = nc.declare_dram_parameter("uT", [D, F], bf16, isOutput=False)
    dT = nc.declare_dram_parameter("dT", [F, D], bf16, isOutput=False)
    # B4 [128, nk/4, 128]: strip i (partitions 32i..32i+31) holds B cols for
    # chunk 4*g+i (lhsT of the 4-packed scale matmul); AT4 [128, w]: A^T
    # replicated on all four strips.
    gB4 = nc.declare_dram_parameter("gB4", [4 * R, KD // 4, P], bf16, isOutput=False)
    uB4 = nc.declare_dram_parameter("uB4", [4 * R, KD // 4, P], bf16, isOutput=False)
    dB4 = nc.declare_dram_parameter("dB4", [4 * R, KF // 4, P], bf16, isOutput=False)
    gAT4 = nc.declare_dram_parameter("gAT4", [4 * R, F], bf16, isOutput=False)
    uAT4 = nc.declare_dram_parameter("uAT4", [4 * R, F], bf16, isOutput=False)
    dAT4 = nc.declare_dram_parameter("dAT4", [4 * R, D], bf16, isOutput=False)
    out = nc.declare_dram_parameter("out", [D, S], bf16, isOutput=True)

    with TileContext(nc) as tc:
        with (
            tc.tile_pool(name="const", bufs=1) as const,
            tc.tile_pool(name="wstream", bufs=12) as wpool,
            tc.tile_pool(name="hbuf", bufs=1) as hpool,
            tc.tile_pool(name="obuf", bufs=4) as opool,
            tc.tile_pool(name="psacc", bufs=1, space="PSUM") as psacc,
            tc.tile_pool(name="pssc", bufs=2, space="PSUM") as pssc,
        ):
            # Startup critical path: sc4(0) needs gB4 + the first 512-col
            # slice of gAT4 (AT factors load as per-pass slices so the first
            # scale matmul isn't gated on the full tensor); job 0's weight
            # chunks lead the scalar ring. x0/x1 ride the tails of the two
            # HWDGE rings; everything else takes the gpsimd SWDGE ring.
            rounded = {}

            def load_factor(nm, dram, eng):
                rt = const.tile(list(dram.shape), bf16, name=f"{nm}r", tag=f"{nm}r")
                eng.dma_start(rt, dram[:])
                rounded[nm] = rt

            def load_at_slice(nm, dram, sl, eng):
                rt = const.tile([4 * R, FG], bf16, name=f"{nm}{sl}", tag=f"{nm}{sl}")
                eng.dma_start(rt, dram[:, sl * FG:(sl + 1) * FG])
                rounded[nm, sl] = rt

            x_sb = [None] * KD

            def load_x_chunk(q, eng):
                xt = const.tile([P, S], bf16, name=f"x{q}", tag=f"x{q}")
                eng.dma_start(xt, x[q * P:(q + 1) * P, :])
                x_sb[q] = xt

            # Up passes run first, so up factors lead the sync ring; job-0
            # weight chunks + x0 lead scalar; everything else on SWDGE.
            load_factor("uB", uB4, nc.sync)
            load_at_slice("uAT", uAT4, 0, nc.sync)
            load_x_chunk(1, nc.gpsimd)
            load_x_chunk(2, nc.gpsimd)
            load_x_chunk(3, nc.gpsimd)

            load_at_slice("uAT", uAT4, 1, nc.gpsimd)
            load_factor("gB", gB4, nc.gpsimd)
            load_at_slice("gAT", gAT4, 0, nc.gpsimd)
            load_at_slice("gAT", gAT4, 1, nc.gpsimd)
            load_factor("dBs", dB4, nc.gpsimd)
            for mg in range(D // FG):
                load_at_slice("dAT", dAT4, mg, nc.gpsimd)

            def xs(kd):
                return x_sb[kd]

            # h = silu(gate) * up, [128, 8, 512] resident. The up passes run
            # FIRST (ACT copies acc into h); the gate passes then silu into a
            # temp on ACT and Pool multiplies h in place (SBUF-only, so it's
            # legal on Pool). DVE does nothing here: it must keep pace with
            # the weight dequants or the 4-packed scale slots split.
            h_sb = hpool.tile([P, KF, S], bf16)

            silu = mybir.ActivationFunctionType.Silu

            def up_finish(fi, fg, acc_fi):
                nc.scalar.copy(h_sb[:, fg * 4 + fi], acc_fi)

            def gate_finish(fi, fg, acc_fi):
                st = opool.tile([P, S], bf16, name="st", tag="st")
                nc.scalar.activation(st, acc_fi, silu)
                nc.gpsimd.tensor_mul(
                    out=h_sb[:, fg * 4 + fi], in0=h_sb[:, fg * 4 + fi],
                    in1=st)

            def down_finish(fi, mg, acc_fi):
                ot = opool.tile([P, S], bf16, name="ot", tag="ot")
                if fi == 3:
                    nc.vector.tensor_copy(out=ot, in_=acc_fi)
                else:
                    nc.scalar.copy(ot, acc_fi)
                weng = nc.sync if fi % 2 == 0 else nc.scalar
                weng.dma_start(out[(mg * 4 + fi) * P:(mg * 4 + fi + 1) * P, :], ot)

            passes = []
            for is_up in (1, 0):
                for fg in range(F // FG):
                    passes.append(dict(
                        wdram=uT if is_up else gT,
                        Bn="uB" if is_up else "gB",
                        An="uAT" if is_up else "gAT",
                        nk=KD, fg=fg, rhs_fn=xs,
                        finish=(lambda fi, acc_fi, fg=fg, is_up=is_up:
                                up_finish(fi, fg, acc_fi) if is_up
                                else gate_finish(fi, fg, acc_fi)),
                    ))
            for mg in range(D // FG):
                passes.append(dict(
                    wdram=dT, Bn="dBs", An="dAT",
                    nk=KF, fg=mg, rhs_fn=lambda kf: h_sb[:, kf],
                    finish=lambda fi, acc_fi, mg=mg: down_finish(fi, mg, acc_fi),
                ))

            # Flat pair-job list. Every pass has an even number of pairs and
            # starts at an even flat index, so (E, E+1) groups for even E
            # never straddle a pass boundary.
            jobs = []
            for pi, ps in enumerate(passes):
                for kp in range(ps["nk"] // 2):
                    jobs.append((pi, kp))
            njobs = len(jobs)

            sc_tiles = {}

            def emit_sc4(E):
                """One 4-packed PE slot computing scale tiles for jobs E and
                E+1 (chunks 4g..4g+3 of pass pi)."""
                pi, kp = jobs[E]
                ps = passes[pi]
                fg = ps["fg"]
                g = kp // 2
                sca = pssc.tile([P, 2, FG], f32, name="sc", tag="sc")
                scb = pssc.tile([P, 2, FG], f32, name="sc", tag="sc")
                for i in range(4):
                    tgt = sca if i < 2 else scb
                    nc.tensor.matmul(
                        tgt[:, i % 2],
                        rounded[ps["Bn"]][i * R:(i + 1) * R, g],
                        rounded[ps["An"], fg][i * R:(i + 1) * R, :],
                        start=True, stop=True,
                        tile_position=(R * i, 0),
                    )
                sc_tiles[E] = sca
                sc_tiles[E + 1] = scb

            wt_tiles = {}
            wr_tiles = {}

            def emit_wt(J):
                """Weight DMA for pair J, issued several jobs ahead of the
                dequant so a slow transfer can't starve the PE. Job 0 splits
                into single-chunk DMAs on opposite rings so the first mains
                start as early as possible."""
                pi, kp = jobs[J]
                ps = passes[pi]
                fg = ps["fg"]
                if J == 0:
                    for j in range(2):
                        wt1 = wpool.tile([P, FG], bf16, name="wt0", tag=f"wt0{j}")
                        nc.scalar.dma_start(
                            wt1,
                            ps["wdram"][(2 * kp + j) * P:(2 * kp + j + 1) * P,
                                        fg * FG:(fg + 1) * FG])
                        wt_tiles[J, j] = wt1
                        if j == 0:
                            # x0 rides between the two job-0 chunks: both are
                            # needed within ~300ns of each other at the start.
                            load_x_chunk(0, nc.scalar)
                    return
                wt2 = wpool.tile([P, 2, FG], bf16, name="wt", tag="wt")
                weng = nc.sync if J % 2 == 1 else nc.scalar
                weng.dma_start(
                    wt2,
                    ps["wdram"][kp * 2 * P:(kp + 1) * 2 * P,
                                fg * FG:(fg + 1) * FG].rearrange(
                                    "(ko p) f -> p ko f", p=P))
                wt_tiles[J] = wt2

            def emit_dequant(J):
                if J == 0:
                    # chunk-granular so j=0 mains only wait on the first
                    # 256 KiB of weights
                    for j in range(2):
                        wr1 = wpool.tile([P, FG], bf16, name="wr0", tag=f"wr0{j}")
                        nc.vector.tensor_mul(
                            out=wr1, in0=wt_tiles.pop((J, j)),
                            in1=sc_tiles[J][:, j])
                        wr_tiles[J, j] = wr1
                    sc_tiles.pop(J)
                    return
                wr2 = wpool.tile([P, 2, FG], bf16, name="wr", tag="wr")
                nc.vector.tensor_mul(out=wr2, in0=wt_tiles.pop(J),
                                     in1=sc_tiles.pop(J))
                wr_tiles[J] = wr2

            def wr_slice(J, j, fi):
                if J == 0:
                    return wr_tiles[J, j][:, fi * P:(fi + 1) * P]
                return wr_tiles[J][:, j, fi * P:(fi + 1) * P]

            def free_wr(J):
                if J == 0:
                    wr_tiles.pop((J, 0))
                    wr_tiles.pop((J, 1))
                else:
                    wr_tiles.pop(J)

            DMA_AHEAD = 8
            for J in range(DMA_AHEAD):
                emit_wt(J)
            emit_sc4(0)
            emit_dequant(0)
            emit_dequant(1)

            cur_acc = {}
            for J, (pi, kp) in enumerate(jobs):
                ps = passes[pi]
                npairs = ps["nk"] // 2
                if kp == 0:
                    cur_acc[pi] = [
                        psacc.tile([P, S], f32, name=f"acc{i}", tag=f"acc{i}")
                        for i in range(4)]
                if pi == 0 and 2 * kp + 5 < KD:
                    # pull the rest of x in just-in-time on the SWDGE ring
                    # (chunk q is first consumed at pair kp=q//2)
                    load_x_chunk(2 * kp + 4, nc.gpsimd)
                    load_x_chunk(2 * kp + 5, nc.gpsimd)
                if J + DMA_AHEAD < njobs:
                    emit_wt(J + DMA_AHEAD)
                if J % 2 == 1 and J + 1 < njobs:
                    # sc4 for the next even group leads mains(J) in the PE
                    # FIFO: its dequants then overlap mains(J)/mains(J+1).
                    emit_sc4(J + 1)
                    emit_dequant(J + 1)
                    emit_dequant(J + 2)
                acc = cur_acc[pi]
                if kp == npairs - 2:
                    continue  # emitted fused with the last pair below
                if kp == npairs - 1:
                    # fi-major tail over the last two pairs: acc[fi] gets its
                    # final accumulation 4 matmuls after acc[fi-1], so the
                    # per-fi epilogue chases the tail.
                    for fi in range(4):
                        for Jt, kpt in ((J - 1, npairs - 2), (J, npairs - 1)):
                            for j in range(2):
                                nc.tensor.matmul(
                                    acc[fi],
                                    wr_slice(Jt, j, fi),
                                    ps["rhs_fn"](2 * kpt + j),
                                    start=False,
                                    stop=(kpt == npairs - 1 and j == 1),
                                )
                        ps["finish"](fi, acc[fi])
                    free_wr(J - 1)
                    free_wr(J)
                    cur_acc.pop(pi)
                    continue
                for j in range(2):
                    for fi in range(4):
                        nc.tensor.matmul(
                            acc[fi],
                            wr_slice(J, j, fi),
                            ps["rhs_fn"](2 * kp + j),
                            start=(kp == 0 and j == 0),
                            stop=False,
                        )
                free_wr(J)
    nc.finalize()
    return nc


def _prep_inputs(x, gate_snapped, gate_scale_A, gate_scale_B,
                 up_snapped, up_scale_A, up_scale_B,
                 down_snapped, down_scale_A, down_scale_B):
    asf = lambda a: np.ascontiguousarray(np.asarray(a, dtype=np.float32))
    bf = ml_dtypes.bfloat16
    x2 = np.ascontiguousarray(np.asarray(x, dtype=np.float32).reshape(D, S)
                              .astype(bf))
    gT_full = asf(gate_snapped).T      # [D, FF] view
    uT_full = asf(up_snapped).T
    dT_full = asf(down_snapped).T      # [FF, D] view

    def pack_B4(Bmat, nk):
        # [R, nk*128] -> [128, nk/4, 128]: strip i holds chunks 4*g+i
        b = np.asarray(Bmat, dtype=np.float32).reshape(R, nk // 4, 4, P)
        o = np.empty((4 * R, nk // 4, P), dtype=bf)
        for i in range(4):
            o[i * R:(i + 1) * R] = b[:, :, i, :].astype(bf)
        return o

    def pack_AT4(Amat):
        # A [w, R] -> A^T [R, w] replicated on all four strips -> [128, w]
        at = np.asarray(Amat, dtype=np.float32).T.astype(bf)
        return np.ascontiguousarray(np.concatenate([at] * 4, axis=0))

    gB_f = np.asarray(gate_scale_B, dtype=np.float32)
    uB_f = np.asarray(up_scale_B, dtype=np.float32)
    dB_f = np.asarray(down_scale_B, dtype=np.float32)
    gA_f = np.asarray(gate_scale_A, dtype=np.float32)
    uA_f = np.asarray(up_scale_A, dtype=np.float32)
    dAT4 = pack_AT4(down_scale_A)      # [128, D]

    in_maps = []
    for c in range(NCORES):
        lo, hi = c * F, (c + 1) * F
        in_maps.append({
            "x": x2,
            "gT": gT_full[:, lo:hi].astype(bf),
            "uT": uT_full[:, lo:hi].astype(bf),
            "dT": dT_full[lo:hi, :].astype(bf),
            "gB4": pack_B4(gB_f, KD),
            "uB4": pack_B4(uB_f, KD),
            "dB4": pack_B4(dB_f[:, lo:hi], KF),
            "gAT4": pack_AT4(gA_f[lo:hi]),
            "uAT4": pack_AT4(uA_f[lo:hi]),
            "dAT4": dAT4,
        })
    return in_maps


def run(trace=False, **inputs):
    if "nc" not in _CACHE:
        _CACHE["nc"] = _build()
    nc = _CACHE["nc"]
    in_maps = _prep_inputs(**inputs)
    try:
        res = run_bass_kernel_spmd(nc, in_maps, list(range(NCORES)), trace=trace)
    except Exception:
        # A transient device flake (NRT_EXEC_UNIT_UNRECOVERABLE) poisons the
        # PJRT client for the process; tearing the backend down and
        # reconnecting recovers it the same way a fresh process does.
        try:
            import jax.extend.backend
            jax.extend.backend.clear_backends()
        except Exception:
            pass
        res = run_bass_kernel_spmd(nc, in_maps, list(range(NCORES)), trace=trace)
    partial = np.zeros((D, S), dtype=np.float32)
    for c in range(NCORES):
        partial += np.asarray(res.results[c]["out"], dtype=np.float32)
    return partial.reshape(1, D, 1, S), res


def kernel(**inputs):
    out, _ = run(trace=False, **inputs)
    return out


if __name__ == "__main__":
    rng = np.random.default_rng(0)
    ins = {
        "x": rng.standard_normal((1, D, 1, S)).astype(np.float32),
        "gate_snapped": (rng.standard_normal((FF, D)) * 0.02).astype(np.float32),
        "gate_scale_A": (rng.standard_normal((FF, R)) * 0.1).astype(np.float32),
        "gate_scale_B": (rng.standard_normal((R, D)) * 0.1).astype(np.float32),
        "up_snapped": (rng.standard_normal((FF, D)) * 0.02).astype(np.float32),
        "up_scale_A": (rng.standard_normal((FF, R)) * 0.1).astype(np.float32),
        "up_scale_B": (rng.standard_normal((R, D)) * 0.1).astype(np.float32),
        "down_snapped": (rng.standard_normal((D, FF)) * 0.02).astype(np.float32),
        "down_scale_A": (rng.standard_normal((D, R)) * 0.1).astype(np.float32),
        "down_scale_B": (rng.standard_normal((R, FF)) * 0.1).astype(np.float32),
    }
    out = kernel(**ins)
    print("kernel ran, out shape", out.shape, "mean abs", np.abs(out).mean())


# revision 17
# speedup vs baseline: 1.0181x; 1.0063x over previous
"""Trainium2 Bass kernel for FFNWithScales (SwiGLU MLP with low-rank dequant scales).

Reference computation (all fp32):
    gate_eff = gate_snapped * (gate_scale_A @ gate_scale_B)       # [8192, 2048]
    up_eff   = up_snapped   * (up_scale_A   @ up_scale_B)         # [8192, 2048]
    down_eff = down_snapped * (down_scale_A @ down_scale_B)       # [2048, 8192]
    h   = silu(gate_eff @ x) * (up_eff @ x)                       # [8192, 512]
    out = down_eff @ h                                            # [2048, 512]

Sharding (8 cores, tensor-parallel on d_ff): core c owns d_ff rows
[c*1024, (c+1)*1024) of gate/up (and the matching columns of down).
Each core computes a full-[2048, 512] partial of the down projection;
partials are summed on the host (the all-reduce step).

Schedule notes (the PE is the pacing engine at ~2.4 GHz / 1 row/cycle bf16):
  - Weights stream as [128, 2, 512] fp32 pairs; a rank-32 scale matmul
    produces the matching scale tile in psum, DVE multiplies (fp32*fp32
    -> bf16), then eight [128,128]x[128,512] bf16 mains accumulate.
  - Scale matmuls for TWO pairs (4 chunks) are packed into ONE PE slot
    via 4-way tile_position quadrants (rows 0/32/64/96, K=32 each), so
    the PE pays the array-reconfig transition half as often.
  - The last two pairs of every pass issue fi-major so each psum
    accumulator finishes ~4 matmuls before the next, letting the
    serial epilogue (ACT silu / Pool muls / copies) chase the tail and
    release accumulators with no pass-boundary bubble. Up-pass muls run
    on Pool only: DVE must stay clear for the next pass's dequants.
  - Startup: job 0's weight DMA is split into two single-chunk DMAs on
    opposite rings with chunk-level dequant/mains so the PE starts
    ~1.5us earlier; factors split across rings; x streams on the
    gpsimd SWDGE ring at chunk granularity.
  - Output is stored bf16 (partials summed fp32 on host): halves store
    traffic and the kernel tail for ~1e-3 extra relative error.
"""

import numpy as np
import ml_dtypes

import concourse.bass as bass
from concourse import bacc
import concourse.mybir as mybir
from concourse.tile import TileContext
from concourse.bass_utils import run_bass_kernel_spmd

P = 128
D = 2048        # d_model
FF = 8192       # d_ff (global)
S = 512         # sequence
R = 32          # rank
NCORES = 8
F = FF // NCORES          # 1024 local d_ff rows
KD = D // P               # 16 d_model chunks
KF = F // P               # 8 local d_ff chunks
FG = 512                  # free-dim group (psum bank width)

f32 = mybir.dt.float32
bf16 = mybir.dt.bfloat16

_CACHE = {}


def _build():
    nc = bacc.Bacc()
    # Everything arrives bf16 in device layout (host staging); the dequant
    # multiply against the fp32 psum scale tile restores scale precision.
    x = nc.declare_dram_parameter("x", [D, S], bf16, isOutput=False)
    gT = nc.declare_dram_parameter("gT", [D, F], bf16, isOutput=False)
    uT = nc.declare_dram_parameter("uT", [D, F], bf16, isOutput=False)
    dT = nc.declare_dram_parameter("dT", [F, D], bf16, isOutput=False)
    # B4 [128, nk/4, 128]: strip i (partitions 32i..32i+31) holds B cols for
    # chunk 4*g+i (lhsT of the 4-packed scale matmul); AT4 [128, w]: A^T
    # replicated on all four strips.
    gB4 = nc.declare_dram_parameter("gB4", [4 * R, KD // 4, P], bf16, isOutput=False)
    uB4 = nc.declare_dram_parameter("uB4", [4 * R, KD // 4, P], bf16, isOutput=False)
    dB4 = nc.declare_dram_parameter("dB4", [4 * R, KF // 4, P], bf16, isOutput=False)
    gAT4 = nc.declare_dram_parameter("gAT4", [4 * R, F], bf16, isOutput=False)
    uAT4 = nc.declare_dram_parameter("uAT4", [4 * R, F], bf16, isOutput=False)
    dAT4 = nc.declare_dram_parameter("dAT4", [4 * R, D], bf16, isOutput=False)
    out = nc.declare_dram_parameter("out", [D, S], bf16, isOutput=True)

    with TileContext(nc) as tc:
        with (
            tc.tile_pool(name="const", bufs=1) as const,
            tc.tile_pool(name="wstream", bufs=12) as wpool,
            tc.tile_pool(name="hbuf", bufs=1) as hpool,
            tc.tile_pool(name="obuf", bufs=4) as opool,
            tc.tile_pool(name="psacc", bufs=1, space="PSUM") as psacc,
            tc.tile_pool(name="pssc", bufs=2, space="PSUM") as pssc,
        ):
            # Startup critical path: sc4(0) needs gB4 + the first 512-col
            # slice of gAT4 (AT factors load as per-pass slices so the first
            # scale matmul isn't gated on the full tensor); job 0's weight
            # chunks lead the scalar ring. x0/x1 ride the tails of the two
            # HWDGE rings; everything else takes the gpsimd SWDGE ring.
            rounded = {}

            def load_factor(nm, dram, eng):
                rt = const.tile(list(dram.shape), bf16, name=f"{nm}r", tag=f"{nm}r")
                eng.dma_start(rt, dram[:])
                rounded[nm] = rt

            def load_at_slice(nm, dram, sl, eng):
                rt = const.tile([4 * R, FG], bf16, name=f"{nm}{sl}", tag=f"{nm}{sl}")
                eng.dma_start(rt, dram[:, sl * FG:(sl + 1) * FG])
                rounded[nm, sl] = rt

            x_sb = [None] * KD

            def load_x_chunk(q, eng):
                xt = const.tile([P, S], bf16, name=f"x{q}", tag=f"x{q}")
                eng.dma_start(xt, x[q * P:(q + 1) * P, :])
                x_sb[q] = xt

            # Up passes run first, so up factors lead the sync ring; job-0
            # weight chunks + x0 lead scalar; everything else on SWDGE.
            load_factor("uB", uB4, nc.sync)
            load_at_slice("uAT", uAT4, 0, nc.sync)
            load_x_chunk(1, nc.gpsimd)
            load_x_chunk(2, nc.gpsimd)
            load_x_chunk(3, nc.gpsimd)

            load_at_slice("uAT", uAT4, 1, nc.gpsimd)
            load_factor("gB", gB4, nc.gpsimd)
            load_at_slice("gAT", gAT4, 0, nc.gpsimd)
            load_at_slice("gAT", gAT4, 1, nc.gpsimd)
            load_factor("dBs", dB4, nc.gpsimd)
            for mg in range(D // FG):
                load_at_slice("dAT", dAT4, mg, nc.gpsimd)

            def xs(kd):
                return x_sb[kd]

            # h = silu(gate) * up, [128, 8, 512] resident. The up passes run
            # FIRST (ACT copies acc into h); the gate passes then silu into a
            # temp on ACT and Pool multiplies h in place (SBUF-only, so it's
            # legal on Pool). DVE does nothing here: it must keep pace with
            # the weight dequants or the 4-packed scale slots split.
            h_sb = hpool.tile([P, KF, S], bf16)

            silu = mybir.ActivationFunctionType.Silu

            def up_finish(fi, fg, acc_fi):
                nc.scalar.copy(h_sb[:, fg * 4 + fi], acc_fi)

            def gate_finish(fi, fg, acc_fi):
                st = opool.tile([P, S], bf16, name="st", tag="st")
                nc.scalar.activation(st, acc_fi, silu)
                nc.gpsimd.tensor_mul(
                    out=h_sb[:, fg * 4 + fi], in0=h_sb[:, fg * 4 + fi],
                    in1=st)

            def down_finish(fi, mg, acc_fi):
                ot = opool.tile([P, S], bf16, name="ot", tag="ot")
                if fi == 3:
                    nc.vector.tensor_copy(out=ot, in_=acc_fi)
                else:
                    nc.scalar.copy(ot, acc_fi)
                weng = nc.sync if fi % 2 == 0 else nc.scalar
                weng.dma_start(out[(mg * 4 + fi) * P:(mg * 4 + fi + 1) * P, :], ot)

            passes = []
            for is_up in (1, 0):
                for fg in range(F // FG):
                    passes.append(dict(
                        wdram=uT if is_up else gT,
                        Bn="uB" if is_up else "gB",
                        An="uAT" if is_up else "gAT",
                        nk=KD, fg=fg, rhs_fn=xs,
                        finish=(lambda fi, acc_fi, fg=fg, is_up=is_up:
                                up_finish(fi, fg, acc_fi) if is_up
                                else gate_finish(fi, fg, acc_fi)),
                    ))
            for mg in range(D // FG):
                passes.append(dict(
                    wdram=dT, Bn="dBs", An="dAT",
                    nk=KF, fg=mg, rhs_fn=lambda kf: h_sb[:, kf],
                    finish=lambda fi, acc_fi, mg=mg: down_finish(fi, mg, acc_fi),
                ))

            # Flat pair-job list. Every pass has an even number of pairs and
            # starts at an even flat index, so (E, E+1) groups for even E
            # never straddle a pass boundary.
            jobs = []
            for pi, ps in enumerate(passes):
                for kp in range(ps["nk"] // 2):
                    jobs.append((pi, kp))
            njobs = len(jobs)

            sc_tiles = {}

            def emit_sc4(E):
                """One 4-packed PE slot computing scale tiles for jobs E and
                E+1 (chunks 4g..4g+3 of pass pi)."""
                pi, kp = jobs[E]
                ps = passes[pi]
                fg = ps["fg"]
                g = kp // 2
                sca = pssc.tile([P, 2, FG], f32, name="sc", tag="sc")
                scb = pssc.tile([P, 2, FG], f32, name="sc", tag="sc")
                for i in range(4):
                    tgt = sca if i < 2 else scb
                    nc.tensor.matmul(
                        tgt[:, i % 2],
                        rounded[ps["Bn"]][i * R:(i + 1) * R, g],
                        rounded[ps["An"], fg][i * R:(i + 1) * R, :],
                        start=True, stop=True,
                        tile_position=(R * i, 0),
                    )
                sc_tiles[E] = sca
                sc_tiles[E + 1] = scb

            wt_tiles = {}
            wr_tiles = {}

            def emit_wt(J):
                """Weight DMA for pair J, issued several jobs ahead of the
                dequant so a slow transfer can't starve the PE. Job 0 splits
                into single-chunk DMAs on opposite rings so the first mains
                start as early as possible."""
                pi, kp = jobs[J]
                ps = passes[pi]
                fg = ps["fg"]
                if J == 0:
                    for j in range(2):
                        wt1 = wpool.tile([P, FG], bf16, name="wt0", tag=f"wt0{j}")
                        nc.scalar.dma_start(
                            wt1,
                            ps["wdram"][(2 * kp + j) * P:(2 * kp + j + 1) * P,
                                        fg * FG:(fg + 1) * FG])
                        wt_tiles[J, j] = wt1
                        if j == 0:
                            # x0 rides between the two job-0 chunks: both are
                            # needed within ~300ns of each other at the start.
                            load_x_chunk(0, nc.scalar)
                    return
                wt2 = wpool.tile([P, 2, FG], bf16, name="wt", tag="wt")
                weng = nc.sync if J % 2 == 1 else nc.scalar
                weng.dma_start(
                    wt2,
                    ps["wdram"][kp * 2 * P:(kp + 1) * 2 * P,
                                fg * FG:(fg + 1) * FG].rearrange(
                                    "(ko p) f -> p ko f", p=P))
                wt_tiles[J] = wt2

            def emit_dequant(J):
                if J == 0:
                    # chunk-granular so j=0 mains only wait on the first
                    # 256 KiB of weights
                    for j in range(2):
                        wr1 = wpool.tile([P, FG], bf16, name="wr0", tag=f"wr0{j}")
                        nc.vector.tensor_mul(
                            out=wr1, in0=wt_tiles.pop((J, j)),
                            in1=sc_tiles[J][:, j])
                        wr_tiles[J, j] = wr1
                    sc_tiles.pop(J)
                    return
                wr2 = wpool.tile([P, 2, FG], bf16, name="wr", tag="wr")
                nc.vector.tensor_mul(out=wr2, in0=wt_tiles.pop(J),
                                     in1=sc_tiles.pop(J))
                wr_tiles[J] = wr2

            def wr_slice(J, j, fi):
                if J == 0:
                    return wr_tiles[J, j][:, fi * P:(fi + 1) * P]
                return wr_tiles[J][:, j, fi * P:(fi + 1) * P]

            def free_wr(J):
                if J == 0:
                    wr_tiles.pop((J, 0))
                    wr_tiles.pop((J, 1))
                else:
                    wr_tiles.pop(J)

            DMA_AHEAD = 8
            for J in range(DMA_AHEAD):
                emit_wt(J)
            emit_sc4(0)
            emit_dequant(0)
            emit_dequant(1)

            cur_acc = {}
            for J, (pi, kp) in enumerate(jobs):
                ps = passes[pi]
                npairs = ps["nk"] // 2
                if kp == 0:
                    cur_acc[pi] = [
                        psacc.tile([P, S], f32, name=f"acc{i}", tag=f"acc{i}")
                        for i in range(4)]
                if pi == 0 and 2 * kp + 5 < KD:
                    # pull the rest of x in just-in-time on the SWDGE ring
                    # (chunk q is first consumed at pair kp=q//2)
                    load_x_chunk(2 * kp + 4, nc.gpsimd)
                    load_x_chunk(2 * kp + 5, nc.gpsimd)
                if J + DMA_AHEAD < njobs:
                    emit_wt(J + DMA_AHEAD)
                if J % 2 == 1 and kp < npairs - 1 and J + 1 < njobs:
                    # sc4 for the next even group leads mains(J) in the PE
                    # FIFO: its dequants then overlap mains(J)/mains(J+1).
                    emit_sc4(J + 1)
                    emit_dequant(J + 1)
                    emit_dequant(J + 2)
                acc = cur_acc[pi]
                if kp == npairs - 2:
                    continue  # emitted fused with the last pair below
                if kp == npairs - 1:
                    # fi-major tail over the last two pairs: acc[fi] gets its
                    # final accumulation 4 matmuls after acc[fi-1], so the
                    # per-fi epilogue chases the tail. The next pass's sc4
                    # goes in the MIDDLE of the tail: any earlier and its
                    # pssc slots (freed by this pass's last two dequants)
                    # aren't recycled yet, splitting the 4-pack.
                    for fi in range(4):
                        if fi == 2 and J + 1 < njobs:
                            emit_sc4(J + 1)
                            emit_dequant(J + 1)
                            emit_dequant(J + 2)
                        for Jt, kpt in ((J - 1, npairs - 2), (J, npairs - 1)):
                            for j in range(2):
                                nc.tensor.matmul(
                                    acc[fi],
                                    wr_slice(Jt, j, fi),
                                    ps["rhs_fn"](2 * kpt + j),
                                    start=False,
                                    stop=(kpt == npairs - 1 and j == 1),
                                )
                        ps["finish"](fi, acc[fi])
                    free_wr(J - 1)
                    free_wr(J)
                    cur_acc.pop(pi)
                    continue
                for j in range(2):
                    for fi in range(4):
                        nc.tensor.matmul(
                            acc[fi],
                            wr_slice(J, j, fi),
                            ps["rhs_fn"](2 * kp + j),
                            start=(kp == 0 and j == 0),
                            stop=False,
                        )
                free_wr(J)
    nc.finalize()
    return nc


def _prep_inputs(x, gate_snapped, gate_scale_A, gate_scale_B,
                 up_snapped, up_scale_A, up_scale_B,
                 down_snapped, down_scale_A, down_scale_B):
    asf = lambda a: np.ascontiguousarray(np.asarray(a, dtype=np.float32))
    bf = ml_dtypes.bfloat16
    x2 = np.ascontiguousarray(np.asarray(x, dtype=np.float32).reshape(D, S)
                              .astype(bf))
    gT_full = asf(gate_snapped).T      # [D, FF] view
    uT_full = asf(up_snapped).T
    dT_full = asf(down_snapped).T      # [FF, D] view

    def pack_B4(Bmat, nk):
        # [R, nk*128] -> [128, nk/4, 128]: strip i holds chunks 4*g+i
        b = np.asarray(Bmat, dtype=np.float32).reshape(R, nk // 4, 4, P)
        o = np.empty((4 * R, nk // 4, P), dtype=bf)
        for i in range(4):
            o[i * R:(i + 1) * R] = b[:, :, i, :].astype(bf)
        return o

    def pack_AT4(Amat):
        # A [w, R] -> A^T [R, w] replicated on all four strips -> [128, w]
        at = np.asarray(Amat, dtype=np.float32).T.astype(bf)
        return np.ascontiguousarray(np.concatenate([at] * 4, axis=0))

    gB_f = np.asarray(gate_scale_B, dtype=np.float32)
    uB_f = np.asarray(up_scale_B, dtype=np.float32)
    dB_f = np.asarray(down_scale_B, dtype=np.float32)
    gA_f = np.asarray(gate_scale_A, dtype=np.float32)
    uA_f = np.asarray(up_scale_A, dtype=np.float32)
    dAT4 = pack_AT4(down_scale_A)      # [128, D]

    in_maps = []
    for c in range(NCORES):
        lo, hi = c * F, (c + 1) * F
        in_maps.append({
            "x": x2,
            "gT": gT_full[:, lo:hi].astype(bf),
            "uT": uT_full[:, lo:hi].astype(bf),
            "dT": dT_full[lo:hi, :].astype(bf),
            "gB4": pack_B4(gB_f, KD),
            "uB4": pack_B4(uB_f, KD),
            "dB4": pack_B4(dB_f[:, lo:hi], KF),
            "gAT4": pack_AT4(gA_f[lo:hi]),
            "uAT4": pack_AT4(uA_f[lo:hi]),
            "dAT4": dAT4,
        })
    return in_maps


def run(trace=False, **inputs):
    if "nc" not in _CACHE:
        _CACHE["nc"] = _build()
    nc = _CACHE["nc"]
    in_maps = _prep_inputs(**inputs)
    try:
        res = run_bass_kernel_spmd(nc, in_maps, list(range(NCORES)), trace=trace)
    except Exception:
        # A transient device flake (NRT_EXEC_UNIT_UNRECOVERABLE) poisons the
        # PJRT client for the process; tearing the backend down and
        # reconnecting recovers it the same way a fresh process does.
        try:
            import jax.extend.backend
            jax.extend.backend.clear_backends()
        except Exception:
            pass
        res = run_bass_kernel_spmd(nc, in_maps, list(range(NCORES)), trace=trace)
    partial = np.zeros((D, S), dtype=np.float32)
    for c in range(NCORES):
        partial += np.asarray(res.results[c]["out"], dtype=np.float32)
    return partial.reshape(1, D, 1, S), res


def kernel(**inputs):
    out, _ = run(trace=False, **inputs)
    return out


if __name__ == "__main__":
    rng = np.random.default_rng(0)
    ins = {
        "x": rng.standard_normal((1, D, 1, S)).astype(np.float32),
        "gate_snapped": (rng.standard_normal((FF, D)) * 0.02).astype(np.float32),
        "gate_scale_A": (rng.standard_normal((FF, R)) * 0.1).astype(np.float32),
        "gate_scale_B": (rng.standard_normal((R, D)) * 0.1).astype(np.float32),
        "up_snapped": (rng.standard_normal((FF, D)) * 0.02).astype(np.float32),
        "up_scale_A": (rng.standard_normal((FF, R)) * 0.1).astype(np.float32),
        "up_scale_B": (rng.standard_normal((R, D)) * 0.1).astype(np.float32),
        "down_snapped": (rng.standard_normal((D, FF)) * 0.02).astype(np.float32),
        "down_scale_A": (rng.standard_normal((D, R)) * 0.1).astype(np.float32),
        "down_scale_B": (rng.standard_normal((R, FF)) * 0.1).astype(np.float32),
    }
    out = kernel(**ins)
    print("kernel ran, out shape", out.shape, "mean abs", np.abs(out).mean())
